# revision 22
# baseline (speedup 1.0000x reference)
"""NashLoss2D on 8 TRN2 NeuronCores — transposed f16 streaming design.

Inputs pred/targ are [10000, 5000] f32; targ has NaNs (missing obs).
Per station (column) j the loss needs four masked row-reductions:
    nansum_j = sum(isnan(targ))          -> cnt = NT - nansum
    s1_j  = sum(tz)        tz = targ | nan->0
    s2_j  = sum(tz^2)
    res_j = sum(dz^2)      dz = (targ - pred) | nan->0
then scalar finalization (mean/sst/valid/per_col), O(NS), host f64.

Design from HW-measured rates under full engine concurrency (SBUF
contention pins every DVE op near 1 cyc/elem — the cost model's 4x/2x
"fast modes" only appear when DVE runs alone), so the winning move is
MINIMUM DVE PASS COUNT:
    DVE  3 passes/tile: nm = (tg_i16 > 0x7C00) with fused accum ->
         nansum; copy_predicated(tg <- 0 where nm) -> tz in place;
         copy_predicated(d <- 0 where nm) -> dz in place.
    Pool 1 pass: d = tg - pr on the RAW inputs (NaN lanes propagate,
         CP zeroes them after) — depends only on the DMAs, so Pool
         starts before the mask exists.
    ACT  3 passes: Square(tz)+acc -> s2, Copy(tz)+acc -> s1
         (tableless), Square(dz)+acc -> res.
Measured ns/tile at [125, 2500] f16: DVE ts+acc 2745, CP 2762,
ACT pass 2562, Pool TT 5625 -> totals/core: DVE ~165, ACT ~154,
Pool ~113, DMA ~70 us.
- Host pre-transposes each core's 625-station slab to station-major
  [625, 10000] f16 (tolerance 2e-2; measured end-to-end error ~1e-6;
  f16 NaNs from np conversion are positive quiet NaNs > 0x7C00 as i16).
- In-place CPs leave only 5 work planes -> bufs=6 hides the
  DMA->Pool/DVE->ACT chain latency across 20 tiles.
"""

import sys
from contextlib import ExitStack

import numpy as np

sys.path.insert(0, "/opt/trn_rl_repo")

import concourse.bass as bass  # noqa: E402
import concourse.tile as tile  # noqa: E402
from concourse import mybir  # noqa: E402
from concourse.bass_utils import run_bass_kernel_spmd  # noqa: E402

NT = 10000  # timesteps
NS = 5000  # stations
NCORES = 8
SC = NS // NCORES  # 625 stations per core
G = 5  # station groups per core (125 partitions each)
P = 125  # partitions (stations per group)
F = 2500  # time-chunk width (free axis)
C = NT // F  # 4 time chunks
NTILE = G * C  # 20 tiles per tensor per core

_NC_CACHE = {}


def _build_nc():
    nc = bass.Bass()
    f16 = mybir.dt.float16
    f32 = mybir.dt.float32
    i16 = mybir.dt.int16
    Act = mybir.ActivationFunctionType
    Op = mybir.AluOpType

    targ = nc.declare_dram_parameter("targ", [SC, NT], f16, isOutput=False)
    pred = nc.declare_dram_parameter("pred", [SC, NT], f16, isOutput=False)
    out = nc.declare_dram_parameter("out", [P, 80], f32, isOutput=True)

    with ExitStack() as ctx:
        tc = ctx.enter_context(tile.TileContext(nc))
        singles = ctx.enter_context(tc.tile_pool(name="singles", bufs=1))
        work = ctx.enter_context(tc.tile_pool(name="work", bufs=6))

        zeros = singles.tile([P, F], f16)
        nc.vector.memset(zeros, 0.0)
        # accum slots split by writing engine (no cross-engine tile deps).
        # dve_acc: nansum at slot; act_acc: s1 at slot, s2 at 20+, res at 40+
        dve_acc = singles.tile([P, 20], f32)
        act_acc = singles.tile([P, 60], f32)

        for g in range(G):
            for c in range(C):
                slot = g * C + c
                tg = work.tile([P, F], f16, tag="tg")
                pr = work.tile([P, F], f16, tag="pr")
                nm = work.tile([P, F], f16, tag="nm")
                d = work.tile([P, F], f16, tag="d")
                sq = work.tile([P, F], f16, tag="sq")

                r0, t0 = g * P, c * F
                nc.sync.dma_start(out=tg, in_=targ[r0 : r0 + P, t0 : t0 + F])
                nc.sync.dma_start(out=pr, in_=pred[r0 : r0 + P, t0 : t0 + F])

                # d = targ - pred on raw inputs; NaN lanes stay NaN and are
                # zeroed by the second copy_predicated below.
                nc.gpsimd.tensor_tensor(d, tg, pr, Op.subtract)
                # nm = 1.0 at NaN lanes (positive quiet f16 NaNs are > 0x7C00
                # as int16); fused accum -> nansum
                nc.vector.tensor_scalar(
                    out=nm, in0=tg[:].bitcast(i16), scalar1=31744.0, op0=Op.is_gt,
                    scalar2=None, op1=Op.add, accum_out=dve_acc[:, slot : slot + 1],
                )
                # tz, dz in place: zero the NaN lanes
                nc.vector.copy_predicated(tg[:], nm[:].bitcast(i16), zeros[:])
                nc.vector.copy_predicated(d[:], nm[:].bitcast(i16), zeros[:])
                nc.scalar.activation(
                    sq, tg, Act.Square, accum_out=act_acc[:, 20 + slot : 21 + slot]
                )
                # s1 = sum(tz) on ACT's third pass; Copy is tableless
                nc.scalar.activation(
                    sq, tg, Act.Copy, accum_out=act_acc[:, slot : slot + 1]
                )
                nc.scalar.activation(
                    sq, d, Act.Square, accum_out=act_acc[:, 40 + slot : 41 + slot]
                )

        nc.sync.dma_start(out=out[:, 0:20], in_=dve_acc)
        nc.sync.dma_start(out=out[:, 20:80], in_=act_acc)

    import bass_rust as _bass_rust
    from concourse.library_overlay import lower_extended_insts

    # raw Bass skips Bacc's codegen_inst_isa_subclasses pass; without it any
    # custom/ISA instruction has empty .instr -> "ISA wrong length"
    lower_extended_insts(nc)
    _bass_rust.generate_event_semaphores(nc)
    return nc


def get_nc():
    if "nc" not in _NC_CACHE:
        _NC_CACHE["nc"] = _build_nc()
    return _NC_CACHE["nc"]


def make_in_maps(pred: np.ndarray, targ: np.ndarray) -> list:
    in_maps = []
    for c in range(NCORES):
        sl = slice(c * SC, (c + 1) * SC)
        in_maps.append(
            {
                "pred": np.ascontiguousarray(pred[:, sl].T).astype(np.float16),
                "targ": np.ascontiguousarray(targ[:, sl].T).astype(np.float16),
            }
        )
    return in_maps


def _unpack(raw: np.ndarray) -> np.ndarray:
    """[125, 80] device accum slots -> [4, SC] stats (cnt, s1, s2, res).

    Column layout: [nansum(0:20) | s1(20:40) | s2(40:60) | res(60:80)],
    slot = g*C + c. Station index = g*125 + p.
    """

    def blk(j):
        b = raw[:, j * 20 : (j + 1) * 20].astype(np.float64)
        return b.reshape(P, G, C).sum(axis=2).T.reshape(SC)  # s = g*125 + p

    return np.stack([NT - blk(0), blk(1), blk(2), blk(3)])


def _finalize(stats: np.ndarray) -> np.ndarray:
    """stats: [4, NS] f64 (cnt, s1, s2, res) -> scalar f32 loss (host, f64)."""
    cnt, s1, s2, res = stats
    cntf = np.maximum(cnt, 1.0)
    mean = s1 / cntf
    sst = s2 - s1 * mean
    valid = (cnt > 10) & (sst != 0.0)
    sst_safe = np.where(valid, np.maximum(sst, 0.0), 1.0)
    per_col = np.where(valid, res / (np.sqrt(sst_safe) + 0.1) ** 2, 0.0)
    n = valid.sum()
    return np.array(per_col.sum() / n, dtype=np.float32)


def finalize_results(results: list) -> np.ndarray:
    stats = np.concatenate([_unpack(r["out"]) for r in results], axis=1)  # [4, NS]
    return _finalize(stats)


def kernel(pred: np.ndarray, targ: np.ndarray) -> np.ndarray:
    nc = get_nc()
    in_maps = make_in_maps(pred, targ)
    try:
        results = run_bass_kernel_spmd(nc, in_maps, list(range(NCORES))).results
    except Exception:
        # tile scheduling is not perfectly deterministic across processes; a
        # rebuild gives a fresh schedule if a rare bad one failed to compile
        _NC_CACHE.clear()
        nc = get_nc()
        results = run_bass_kernel_spmd(nc, in_maps, list(range(NCORES))).results
    return finalize_results(results)


# revision 25
# speedup vs baseline: 1.1071x; 1.1071x over previous
"""NashLoss2D on 8 TRN2 NeuronCores — transposed f16 streaming design.

Inputs pred/targ are [10000, 5000] f32; targ has NaNs (missing obs).
Per station (column) j the loss needs four masked row-reductions:
    nansum_j = sum(isnan(targ))          -> cnt = NT - nansum
    s1_j  = sum(tz)        tz = targ | nan->0
    s2_j  = sum(tz^2)
    res_j = sum(dz^2)      dz = (targ - pred) | nan->0
then scalar finalization (mean/sst/valid/per_col), O(NS), host f64.

Design from HW-measured rates under full engine concurrency (SBUF
contention pins every DVE op near 1 cyc/elem — the cost model's 4x/2x
"fast modes" only appear when DVE runs alone), so the winning move is
MINIMUM DVE PASS COUNT:
    DVE  3 passes/tile: nm = (tg_i16 > 0x7C00) with fused accum ->
         nansum; copy_predicated(tg <- 0 where nm) -> tz in place;
         copy_predicated(d <- 0 where nm) -> dz in place.
    Pool 1 pass: d = tg - pr on the RAW inputs (NaN lanes propagate,
         CP zeroes them after) — depends only on the DMAs, so Pool
         starts before the mask exists.
    ACT  3 passes: Square(tz)+acc -> s2, Copy(tz)+acc -> s1
         (tableless), Square(dz)+acc -> res.
Measured ns/tile at [125, 2500] f16: DVE ts+acc 2745, CP 2762,
ACT pass 2562, Pool TT 5625 -> totals/core: DVE ~165, ACT ~154,
Pool ~113, DMA ~70 us.
- Host pre-transposes each core's 625-station slab to station-major
  [625, 10000] f16 (tolerance 2e-2; measured end-to-end error ~1e-6;
  f16 NaNs from np conversion are positive quiet NaNs > 0x7C00 as i16).
- In-place CPs leave only 5 work planes -> bufs=6 hides the
  DMA->Pool/DVE->ACT chain latency across 20 tiles.
"""

import sys
from contextlib import ExitStack

import numpy as np

sys.path.insert(0, "/opt/trn_rl_repo")

import concourse.bass as bass  # noqa: E402
import concourse.tile as tile  # noqa: E402
from concourse import mybir  # noqa: E402
from concourse.bass_utils import run_bass_kernel_spmd  # noqa: E402

NT = 10000  # timesteps
NS = 5000  # stations
NCORES = 8
SC = NS // NCORES  # 625 stations per core
G = 5  # station groups per core (125 partitions each)
P = 125  # partitions (stations per group)
F = 2500  # time-chunk width (free axis)
C = NT // F  # 4 time chunks
NTILE = G * C  # 20 tiles per tensor per core

_NC_CACHE = {}


def _build_nc():
    nc = bass.Bass()
    f16 = mybir.dt.float16
    f32 = mybir.dt.float32
    i16 = mybir.dt.int16
    Act = mybir.ActivationFunctionType
    Op = mybir.AluOpType

    targ = nc.declare_dram_parameter("targ", [SC, NT], f16, isOutput=False)
    pred = nc.declare_dram_parameter("pred", [SC, NT], f16, isOutput=False)
    out = nc.declare_dram_parameter("out", [P, 80], f32, isOutput=True)

    with ExitStack() as ctx:
        tc = ctx.enter_context(tile.TileContext(nc))
        singles = ctx.enter_context(tc.tile_pool(name="singles", bufs=1))
        work = ctx.enter_context(tc.tile_pool(name="work", bufs=6))

        # stride-0 broadcast views cut SBUF traffic (the shared-SBUF
        # bandwidth is the real ceiling): the CP zero-source reads one
        # element per partition instead of a full plane.
        zeros1 = singles.tile([P, 1], f16)
        nc.vector.memset(zeros1, 0.0)
        zerosb = zeros1.broadcast_to([P, F])
        # accum slots split by writing engine (no cross-engine tile deps).
        # dve_acc: nansum at slot; act_acc: s1 at slot, s2 at 20+, res at 40+
        dve_acc = singles.tile([P, 20], f32)
        act_acc = singles.tile([P, 60], f32)

        for g in range(G):
            for c in range(C):
                slot = g * C + c
                tg = work.tile([P, F], f16, tag="tg")
                pr = work.tile([P, F], f16, tag="pr")
                nm = work.tile([P, F], f16, tag="nm")
                d = work.tile([P, F], f16, tag="d")
                # ACT outs are don't-care: a [P,1] broadcast sinks the writes
                sq = work.tile([P, 1], f16, tag="sq")
                sqb = sq.broadcast_to([P, F])

                r0, t0 = g * P, c * F
                nc.sync.dma_start(out=tg, in_=targ[r0 : r0 + P, t0 : t0 + F])
                nc.sync.dma_start(out=pr, in_=pred[r0 : r0 + P, t0 : t0 + F])

                # d = targ - pred on raw inputs; NaN lanes stay NaN and are
                # zeroed by the second copy_predicated below.
                nc.gpsimd.tensor_tensor(d, tg, pr, Op.subtract)
                # nm = 1.0 at NaN lanes (positive quiet f16 NaNs are > 0x7C00
                # as int16); fused accum -> nansum
                nc.vector.tensor_scalar(
                    out=nm, in0=tg[:].bitcast(i16), scalar1=31744.0, op0=Op.is_gt,
                    scalar2=None, op1=Op.add, accum_out=dve_acc[:, slot : slot + 1],
                )
                # tz, dz in place: zero the NaN lanes
                nc.vector.copy_predicated(tg[:], nm[:].bitcast(i16), zerosb)
                nc.vector.copy_predicated(d[:], nm[:].bitcast(i16), zerosb)
                nc.scalar.activation(
                    sqb, tg, Act.Square, accum_out=act_acc[:, 20 + slot : 21 + slot]
                )
                # s1 = sum(tz) on ACT's third pass; Copy is tableless
                nc.scalar.activation(
                    sqb, tg, Act.Copy, accum_out=act_acc[:, slot : slot + 1]
                )
                nc.scalar.activation(
                    sqb, d, Act.Square, accum_out=act_acc[:, 40 + slot : 41 + slot]
                )

        nc.sync.dma_start(out=out[:, 0:20], in_=dve_acc)
        nc.sync.dma_start(out=out[:, 20:80], in_=act_acc)

    import bass_rust as _bass_rust
    from concourse.library_overlay import lower_extended_insts

    # raw Bass skips Bacc's codegen_inst_isa_subclasses pass; without it any
    # custom/ISA instruction has empty .instr -> "ISA wrong length"
    lower_extended_insts(nc)
    _bass_rust.generate_event_semaphores(nc)
    return nc


def get_nc():
    if "nc" not in _NC_CACHE:
        _NC_CACHE["nc"] = _build_nc()
    return _NC_CACHE["nc"]


def make_in_maps(pred: np.ndarray, targ: np.ndarray) -> list:
    in_maps = []
    for c in range(NCORES):
        sl = slice(c * SC, (c + 1) * SC)
        in_maps.append(
            {
                "pred": np.ascontiguousarray(pred[:, sl].T).astype(np.float16),
                "targ": np.ascontiguousarray(targ[:, sl].T).astype(np.float16),
            }
        )
    return in_maps


def _unpack(raw: np.ndarray) -> np.ndarray:
    """[125, 80] device accum slots -> [4, SC] stats (cnt, s1, s2, res).

    Column layout: [nansum(0:20) | s1(20:40) | s2(40:60) | res(60:80)],
    slot = g*C + c. Station index = g*125 + p.
    """

    def blk(j):
        b = raw[:, j * 20 : (j + 1) * 20].astype(np.float64)
        return b.reshape(P, G, C).sum(axis=2).T.reshape(SC)  # s = g*125 + p

    return np.stack([NT - blk(0), blk(1), blk(2), blk(3)])


def _finalize(stats: np.ndarray) -> np.ndarray:
    """stats: [4, NS] f64 (cnt, s1, s2, res) -> scalar f32 loss (host, f64)."""
    cnt, s1, s2, res = stats
    cntf = np.maximum(cnt, 1.0)
    mean = s1 / cntf
    sst = s2 - s1 * mean
    valid = (cnt > 10) & (sst != 0.0)
    sst_safe = np.where(valid, np.maximum(sst, 0.0), 1.0)
    per_col = np.where(valid, res / (np.sqrt(sst_safe) + 0.1) ** 2, 0.0)
    n = valid.sum()
    return np.array(per_col.sum() / n, dtype=np.float32)


def finalize_results(results: list) -> np.ndarray:
    stats = np.concatenate([_unpack(r["out"]) for r in results], axis=1)  # [4, NS]
    return _finalize(stats)


def kernel(pred: np.ndarray, targ: np.ndarray) -> np.ndarray:
    nc = get_nc()
    in_maps = make_in_maps(pred, targ)
    try:
        results = run_bass_kernel_spmd(nc, in_maps, list(range(NCORES))).results
    except Exception:
        # tile scheduling is not perfectly deterministic across processes; a
        # rebuild gives a fresh schedule if a rare bad one failed to compile
        _NC_CACHE.clear()
        nc = get_nc()
        results = run_bass_kernel_spmd(nc, in_maps, list(range(NCORES))).results
    return finalize_results(results)


# revision 26
# speedup vs baseline: 1.1181x; 1.0099x over previous
"""NashLoss2D on 8 TRN2 NeuronCores — transposed f16 streaming design.

Inputs pred/targ are [10000, 5000] f32; targ has NaNs (missing obs).
Per station (column) j the loss needs four masked row-reductions:
    nansum_j = sum(isnan(targ))          -> cnt = NT - nansum
    s1_j  = sum(tz)        tz = targ | nan->0
    s2_j  = sum(tz^2)
    res_j = sum(dz^2)      dz = (targ - pred) | nan->0
then scalar finalization (mean/sst/valid/per_col), O(NS), host f64.

Design from HW-measured rates under full engine concurrency (SBUF
contention pins every DVE op near 1 cyc/elem — the cost model's 4x/2x
"fast modes" only appear when DVE runs alone), so the winning move is
MINIMUM DVE PASS COUNT:
    DVE  3 passes/tile: nm = (tg_i16 > 0x7C00) with fused accum ->
         nansum; copy_predicated(tg <- 0 where nm) -> tz in place;
         copy_predicated(d <- 0 where nm) -> dz in place.
    Pool 1 pass: d = tg - pr on the RAW inputs (NaN lanes propagate,
         CP zeroes them after) — depends only on the DMAs, so Pool
         starts before the mask exists.
    ACT  3 passes: Square(tz)+acc -> s2, Copy(tz)+acc -> s1
         (tableless), Square(dz)+acc -> res.
Measured ns/tile at [125, 2500] f16: DVE ts+acc 2745, CP 2762,
ACT pass 2562, Pool TT 5625 -> totals/core: DVE ~165, ACT ~154,
Pool ~113, DMA ~70 us.
- Host pre-transposes each core's 625-station slab to station-major
  [625, 10000] f16 (tolerance 2e-2; measured end-to-end error ~1e-6;
  f16 NaNs from np conversion are positive quiet NaNs > 0x7C00 as i16).
- In-place CPs leave only 5 work planes -> bufs=6 hides the
  DMA->Pool/DVE->ACT chain latency across 20 tiles.
"""

import sys
from contextlib import ExitStack

import numpy as np

sys.path.insert(0, "/opt/trn_rl_repo")

import concourse.bass as bass  # noqa: E402
import concourse.tile as tile  # noqa: E402
from concourse import mybir  # noqa: E402
from concourse.bass_utils import run_bass_kernel_spmd  # noqa: E402

NT = 10000  # timesteps
NS = 5000  # stations
NCORES = 8
SC = NS // NCORES  # 625 stations per core
G = 5  # station groups per core (125 partitions each)
P = 125  # partitions (stations per group)
F = 2500  # time-chunk width (free axis)
C = NT // F  # 4 time chunks
NTILE = G * C  # 20 tiles per tensor per core

_NC_CACHE = {}


def _build_nc():
    nc = bass.Bass()
    f16 = mybir.dt.float16
    f32 = mybir.dt.float32
    i16 = mybir.dt.int16
    Act = mybir.ActivationFunctionType
    Op = mybir.AluOpType

    targ = nc.declare_dram_parameter("targ", [SC, NT], f16, isOutput=False)
    pred = nc.declare_dram_parameter("pred", [SC, NT], f16, isOutput=False)
    out = nc.declare_dram_parameter("out", [P, 80], f32, isOutput=True)

    with ExitStack() as ctx:
        tc = ctx.enter_context(tile.TileContext(nc))
        singles = ctx.enter_context(tc.tile_pool(name="singles", bufs=1))
        work = ctx.enter_context(tc.tile_pool(name="work", bufs=6))

        # stride-0 broadcast views cut SBUF traffic (the shared-SBUF
        # bandwidth is the real ceiling): the CP zero-source reads one
        # element per partition instead of a full plane.
        zeros1 = singles.tile([P, 1], f16)
        nc.vector.memset(zeros1, 0.0)
        zerosb = zeros1.broadcast_to([P, F])
        # accum slots split by writing engine (no cross-engine tile deps).
        # dve_acc: nansum at slot; act_acc: s1 at slot, s2 at 20+, res at 40+
        dve_acc = singles.tile([P, 20], f32)
        act_acc = singles.tile([P, 60], f32)

        for g in range(G):
            for c in range(C):
                slot = g * C + c
                tg = work.tile([P, F], f16, tag="tg")
                pr = work.tile([P, F], f16, tag="pr")
                nm = work.tile([P, F], f16, tag="nm")
                d = work.tile([P, F], f16, tag="d")
                # ACT outs are don't-care: a [P,1] broadcast sinks the writes
                sq = work.tile([P, 1], f16, tag="sq")
                sqb = sq.broadcast_to([P, F])

                r0, t0 = g * P, c * F
                # split input loads across two HWDGE queues: descriptor
                # generation (~2.8 us per 125-desc DMA) serializes per queue
                # and was pacing the pipeline on Sync alone
                nc.sync.dma_start(out=tg, in_=targ[r0 : r0 + P, t0 : t0 + F])
                nc.scalar.dma_start(out=pr, in_=pred[r0 : r0 + P, t0 : t0 + F])

                # d = targ - pred on raw inputs; NaN lanes stay NaN and are
                # zeroed by the second copy_predicated below.
                nc.gpsimd.tensor_tensor(d, tg, pr, Op.subtract)
                # nm = 1.0 at NaN lanes (positive quiet f16 NaNs are > 0x7C00
                # as int16); fused accum -> nansum
                nc.vector.tensor_scalar(
                    out=nm, in0=tg[:].bitcast(i16), scalar1=31744.0, op0=Op.is_gt,
                    scalar2=None, op1=Op.add, accum_out=dve_acc[:, slot : slot + 1],
                )
                # tz, dz in place: zero the NaN lanes
                nc.vector.copy_predicated(tg[:], nm[:].bitcast(i16), zerosb)
                nc.vector.copy_predicated(d[:], nm[:].bitcast(i16), zerosb)
                nc.scalar.activation(
                    sqb, tg, Act.Square, accum_out=act_acc[:, 20 + slot : 21 + slot]
                )
                # s1 = sum(tz) on ACT's third pass; Copy is tableless
                nc.scalar.activation(
                    sqb, tg, Act.Copy, accum_out=act_acc[:, slot : slot + 1]
                )
                nc.scalar.activation(
                    sqb, d, Act.Square, accum_out=act_acc[:, 40 + slot : 41 + slot]
                )

        nc.sync.dma_start(out=out[:, 0:20], in_=dve_acc)
        nc.sync.dma_start(out=out[:, 20:80], in_=act_acc)

    import bass_rust as _bass_rust
    from concourse.library_overlay import lower_extended_insts

    # raw Bass skips Bacc's codegen_inst_isa_subclasses pass; without it any
    # custom/ISA instruction has empty .instr -> "ISA wrong length"
    lower_extended_insts(nc)
    _bass_rust.generate_event_semaphores(nc)
    return nc


def get_nc():
    if "nc" not in _NC_CACHE:
        _NC_CACHE["nc"] = _build_nc()
    return _NC_CACHE["nc"]


def make_in_maps(pred: np.ndarray, targ: np.ndarray) -> list:
    in_maps = []
    for c in range(NCORES):
        sl = slice(c * SC, (c + 1) * SC)
        in_maps.append(
            {
                "pred": np.ascontiguousarray(pred[:, sl].T).astype(np.float16),
                "targ": np.ascontiguousarray(targ[:, sl].T).astype(np.float16),
            }
        )
    return in_maps


def _unpack(raw: np.ndarray) -> np.ndarray:
    """[125, 80] device accum slots -> [4, SC] stats (cnt, s1, s2, res).

    Column layout: [nansum(0:20) | s1(20:40) | s2(40:60) | res(60:80)],
    slot = g*C + c. Station index = g*125 + p.
    """

    def blk(j):
        b = raw[:, j * 20 : (j + 1) * 20].astype(np.float64)
        return b.reshape(P, G, C).sum(axis=2).T.reshape(SC)  # s = g*125 + p

    return np.stack([NT - blk(0), blk(1), blk(2), blk(3)])


def _finalize(stats: np.ndarray) -> np.ndarray:
    """stats: [4, NS] f64 (cnt, s1, s2, res) -> scalar f32 loss (host, f64)."""
    cnt, s1, s2, res = stats
    cntf = np.maximum(cnt, 1.0)
    mean = s1 / cntf
    sst = s2 - s1 * mean
    valid = (cnt > 10) & (sst != 0.0)
    sst_safe = np.where(valid, np.maximum(sst, 0.0), 1.0)
    per_col = np.where(valid, res / (np.sqrt(sst_safe) + 0.1) ** 2, 0.0)
    n = valid.sum()
    return np.array(per_col.sum() / n, dtype=np.float32)


def finalize_results(results: list) -> np.ndarray:
    stats = np.concatenate([_unpack(r["out"]) for r in results], axis=1)  # [4, NS]
    return _finalize(stats)


def kernel(pred: np.ndarray, targ: np.ndarray) -> np.ndarray:
    nc = get_nc()
    in_maps = make_in_maps(pred, targ)
    try:
        results = run_bass_kernel_spmd(nc, in_maps, list(range(NCORES))).results
    except Exception:
        # tile scheduling is not perfectly deterministic across processes; a
        # rebuild gives a fresh schedule if a rare bad one failed to compile
        _NC_CACHE.clear()
        nc = get_nc()
        results = run_bass_kernel_spmd(nc, in_maps, list(range(NCORES))).results
    return finalize_results(results)
